# revision 7
# baseline (speedup 1.0000x reference)
"""GAT layer kernel for Trainium2, 8 NeuronCores.

Problem: nn_GATLayer (B=4, N=2048, IN_F=256, OUT_F=64, H=8).

Key algebra: softmax over j of (src[b,i,h] + dst[b,j,h]) masked by adj[b,i,j].
src[b,i,h] is constant over j, so it cancels in the softmax:
    out[b,i,(h,f)] = (adj[b,i,:] @ g[b,:, (h,f)]) / (adj[b,i,:] @ e[b,:,h])
with hfeat = x@W (per-head features), dst[j,h] = x[j,:] @ (W . attn_dst)[:,h],
e = exp(dst), g = e * hfeat.  attn_src is mathematically irrelevant.

Sharding: 8 cores = 4 batches x 2 row-halves of i (softmax is over j only,
so row-sharding of i needs no communication).

All-bf16 datapath (host casts x/W/adj to bf16; attn_dst folded into the
weight on the host).  Schedule notes:
  - PE pre-warm: dummy matmuls on a memset tile bridge the tensor engine
    p-state ramp while the first input DMAs land.
  - Projection jc-loop is interleaved with the first aggregation wave
    (ics 0-3) so PE never sits behind the DVE g-multiply.
  - dst logits live in ONE psum bank ([128,16,8] slices, single
    start=True zeroes the bank); denominators for all 8 ics live in ONE
    psum bank the same way.  4 numerator banks + 2 rotating hfeat banks
    + dst + den = exactly 8 psum banks.
  - bf16 stores, upcast to f32 on the host.
"""

import numpy as np
import ml_dtypes

B, N, IN_F, OUT_F, H = 4, 2048, 256, 64, 8
HF = H * OUT_F            # 512 concat features
NCORES = 8
ROWS = B * N // NCORES    # 1024 destination rows per core
P = 128
IC = ROWS // P            # 8 i-chunks per core
JC = N // P               # 16 j-chunks
KC = IN_F // P            # 2 k-chunks
JG = 8                    # adjT DMA groups (JC/JG j-chunks each)
WAVE = 4                  # i-chunks in the first (interleaved) wave
NDUMMY = 8                # PE p-state pre-warm matmuls

BF16 = ml_dtypes.bfloat16

_CACHE = {}


def _bcast_last(ap, n):
    """View ap with an extra innermost broadcast (stride-0) dim of size n."""
    ap2 = ap.unsqueeze(len(ap.shape))
    return ap2.broadcast_to(tuple(ap.shape) + (n,))


def _build():
    import concourse.mybir as mybir
    import concourse.tile as tile
    from concourse import bacc

    f32 = mybir.dt.float32
    bf = mybir.dt.bfloat16
    MULT = mybir.AluOpType.mult

    nc = bacc.Bacc(trn_type="TRN2", debug=False, target_bir_lowering=False)

    adjt_d = nc.dram_tensor("adjt", [N, ROWS], bf, kind="ExternalInput")
    w_d = nc.dram_tensor("w", [P, KC * HF], bf, kind="ExternalInput")
    wdst_d = nc.dram_tensor("wdst", [P, KC * H], bf, kind="ExternalInput")
    xt_d = nc.dram_tensor("xt", [P, 2 * KC * (N // 2)], bf, kind="ExternalInput")
    out_ds = [
        nc.dram_tensor(f"out{q}", [P, HF], bf, kind="ExternalOutput")
        for q in range(IC)
    ]

    with tile.TileContext(nc) as tc:
        with (
            tc.tile_pool(name="setup", bufs=1) as setup,
            tc.tile_pool(name="gpool", bufs=1) as gpool,
            tc.tile_pool(name="scratch", bufs=2) as scr,
            tc.tile_pool(name="adjT", bufs=1) as adjTp,
            tc.tile_pool(name="warm", bufs=1) as warmp,
            tc.tile_pool(name="ps_num", bufs=4, space="PSUM") as psnum,
            tc.tile_pool(name="ps_h", bufs=2, space="PSUM") as psh,
            tc.tile_pool(name="ps_dst", bufs=1, space="PSUM") as psdst,
            tc.tile_pool(name="ps_den", bufs=1, space="PSUM") as psden,
            tc.tile_pool(name="nsbp", bufs=4) as nsbp,
        ):
            # --- PE pre-warm: memset a junk tile, matmul it repeatedly.
            # Dummies write into pF0, whose first real matmul re-starts the
            # accumulation group, so no extra psum bank is consumed. ---
            pFs = [psnum.tile([P, HF], f32, tag="num", name=f"pF0_{k}")
                   for k in range(WAVE)]
            junk = warmp.tile([P, P], bf)
            nc.vector.memset(junk[:], 0.0)
            junk_rhs = junk[:, 0:P].unsqueeze(1).broadcast_to((P, 4, P))
            for _ in range(NDUMMY):
                nc.tensor.matmul(pFs[0][:], junk[:], junk_rhs,
                                 start=True, stop=True, skip_group_check=True)

            # --- input streams, ordered by first consumption ---
            w_sb = setup.tile([P, KC, HF], bf)
            w_v = w_d.rearrange("p (kc n) -> p kc n", kc=KC)
            for kc in range(KC):
                nc.gpsimd.dma_start(w_sb[:, kc], w_v[:, kc])
            wdst_sb = setup.tile([P, KC, H], bf)
            nc.sync.dma_start(
                wdst_sb[:], wdst_d.rearrange("p (kc h) -> p kc h", kc=KC))

            # x^T in 4 j-slabs of 512; adj^T in 8 groups of 2 j-chunks,
            # interleaved so each lands just before PE consumes it
            xT_sb = setup.tile([P, 2, KC, N // 2], bf)
            xt_v = xt_d.rearrange("p (jh kc j) -> p jh kc j", jh=2, kc=KC)
            nj = JC // JG
            adjT_g = []
            for G in range(JG):
                adjT_g.append(adjTp.tile([P, nj, ROWS], bf, tag=f"adjt{G}",
                                         name=f"adjt{G}"))

            def load_slab(s):
                jh, j0 = divmod(s * (N // 4), N // 2)
                nc.gpsimd.dma_start(
                    xT_sb[:, jh, :, j0:j0 + N // 4],
                    xt_v[:, jh, :, j0:j0 + N // 4])

            def load_adjt(G):
                nc.gpsimd.dma_start(
                    adjT_g[G][:],
                    adjt_d[G * nj * P:(G + 1) * nj * P, :].rearrange(
                        "(jc jp) i -> jp jc i", jp=P),
                )

            load_slab(0)
            load_adjt(0)
            load_slab(1)
            load_adjt(1)
            load_slab(2)
            load_slab(3)
            for G in range(2, JG):
                load_adjt(G)

            def xT(kc, jc):
                jh, j0 = divmod(jc * P, N // 2)
                return xT_sb[:, jh, kc, j0:j0 + P]

            # g_sb[jp, jc, :] = e * hfeat ; e_sb[jp, jc, :] = exp(dst)
            g_sb = gpool.tile([P, JC, HF], bf)
            e_sb = gpool.tile([P, JC, H], bf)
            # dst logits: one bank, 16 jc slices; single bank-zeroing start
            pdst = psdst.tile([P, JC, H], f32)
            # denominators: one bank, 8 ic slices; single bank-zeroing start
            pden = psden.tile([P, IC, H], f32)

            def proj(jc):
                ph = psh.tile([P, HF], f32, tag="hfeat")
                for kc in range(KC):
                    nc.tensor.matmul(
                        ph[:], xT(kc, jc), w_sb[:, kc, :],
                        start=(kc == 0), stop=(kc == KC - 1),
                    )
                for kc in range(KC):
                    nc.tensor.matmul(
                        pdst[:, jc, :], xT(kc, jc), wdst_sb[:, kc, :],
                        start=(jc == 0 and kc == 0), stop=(kc == KC - 1),
                        skip_group_check=True,
                    )
                e_cols = e_sb[:, jc, :]
                nc.scalar.activation(
                    e_cols, pdst[:, jc, :], mybir.ActivationFunctionType.Exp
                )
                o3 = g_sb[:, jc, :].rearrange("p (h f) -> p h f", h=H)
                h3 = ph[:].rearrange("p (h f) -> p h f", h=H)
                e3 = _bcast_last(e_cols, OUT_F)
                nc.vector.tensor_tensor(o3, h3, e3, op=MULT)

            def mm_pair(pF, ic, jc):
                G, t = divmod(jc, nj)
                lhs = adjT_g[G][:, t, ic * P:(ic + 1) * P]
                nc.tensor.matmul(
                    pF[:], lhs, g_sb[:, jc, :],
                    start=(jc == 0), stop=(jc == JC - 1),
                )
                nc.tensor.matmul(
                    pden[:, ic, :], lhs, e_sb[:, jc, :],
                    start=(ic == 0 and jc == 0), stop=(jc == JC - 1),
                    skip_group_check=True,
                )

            def finalize(ic, pF):
                rc = scr.tile([P, H], f32, tag="rc")
                nc.vector.reciprocal(rc[:], pden[:, ic, :])
                nsb = nsbp.tile([P, HF], bf, tag="nsb")
                n3 = nsb[:].rearrange("p (h f) -> p h f", h=H)
                p3 = pF[:].rearrange("p (h f) -> p h f", h=H)
                r3 = _bcast_last(rc[:], OUT_F)
                nc.vector.tensor_tensor(n3, p3, r3, op=MULT)
                nc.sync.dma_start(out_ds[ic][:, :], nsb[:])

            # --- wave 0: projection interleaved with ics 0-3, jc-major ---
            for jc in range(JC):
                proj(jc)
                for k in range(WAVE):
                    mm_pair(pFs[k], k, jc)
            for k in range(WAVE):
                finalize(k, pFs[k])

            # --- wave 1: ics 4-7, ic-major with inline finalize ---
            for k in range(WAVE):
                ic = WAVE + k
                pF = psnum.tile([P, HF], f32, tag="num", name=f"pF1_{k}")
                for jc in range(JC):
                    mm_pair(pF, ic, jc)
                finalize(ic, pF)

    nc.compile()
    return nc


def _get_nc():
    if "nc" not in _CACHE:
        _CACHE["nc"] = _build()
    return _CACHE["nc"]


def _make_in_maps(x, adj, weight, attn_dst):
    x = np.ascontiguousarray(np.asarray(x), dtype=np.float32)
    adj = np.asarray(adj)
    weight = np.ascontiguousarray(np.asarray(weight), dtype=np.float32)
    attn_dst = np.ascontiguousarray(np.asarray(attn_dst), dtype=np.float32)

    # fold attn_dst into the weight: wdst[k, h] = sum_f W[k, h*64+f]*adst[h, f]
    wdst = (weight.reshape(IN_F, H, OUT_F) * attn_dst[None]).sum(-1)

    w_kp = np.ascontiguousarray(
        weight.reshape(KC, P, HF).transpose(1, 0, 2).reshape(P, KC * HF)
    ).astype(BF16)
    wdst_kp = np.ascontiguousarray(
        wdst.reshape(KC, P, H).transpose(1, 0, 2).reshape(P, KC * H)
    ).astype(BF16)

    in_maps = []
    for core in range(NCORES):
        b = core // 2
        half = core % 2
        # xt layout [p, jh, kc, j']: x[b][jh*1024 + j', kc*128 + p]
        xt = x[b].T.reshape(KC, P, 2, N // 2)          # [kc, p, jh, j']
        xt_kp = np.ascontiguousarray(
            xt.transpose(1, 2, 0, 3).reshape(P, 2 * KC * (N // 2))
        ).astype(BF16)
        adjt = adj[b].T[:, half * ROWS:(half + 1) * ROWS]  # [N, ROWS]
        in_maps.append({
            "adjt": np.ascontiguousarray(adjt, dtype=np.float32).astype(BF16),
            "w": w_kp,
            "wdst": wdst_kp,
            "xt": xt_kp,
        })
    return in_maps


def _run_device(in_maps):
    from concourse import bass_utils

    nc = _get_nc()
    res = bass_utils.run_bass_kernel_spmd(
        nc, in_maps, core_ids=list(range(NCORES)))
    return [dict(r) for r in res.results]


def _run_device_subprocess(in_maps):
    """Fresh-process fallback: a wedged accelerator surfaces as
    NRT_EXEC_UNIT_UNRECOVERABLE and poisons the in-process PJRT client;
    a new process gets a fresh axon session and a reset device."""
    import os
    import pickle
    import subprocess
    import sys
    import tempfile

    d = tempfile.mkdtemp(prefix="gat_kernel_")
    inp = os.path.join(d, "in.pkl")
    outp = os.path.join(d, "out.pkl")
    with open(inp, "wb") as f:
        pickle.dump(in_maps, f)
    code = (
        "import pickle, sys\n"
        f"sys.path.insert(0, {os.path.dirname(os.path.abspath(__file__))!r})\n"
        "import kernel\n"
        f"in_maps = pickle.load(open({inp!r}, 'rb'))\n"
        f"pickle.dump(kernel._run_device(in_maps), open({outp!r}, 'wb'))\n"
    )
    env = dict(os.environ, GAT_KERNEL_SUBPROC="1")
    subprocess.run([sys.executable, "-c", code], check=True, env=env,
                   timeout=1200)
    with open(outp, "rb") as f:
        return pickle.load(f)


def kernel(x, adj, weight, attn_src, attn_dst):
    import os
    import time

    in_maps = _make_in_maps(x, adj, weight, attn_dst)
    try:
        results = _run_device(in_maps)
    except Exception:
        if os.environ.get("GAT_KERNEL_SUBPROC") == "1":
            raise
        time.sleep(2)
        results = _run_device_subprocess(in_maps)

    out = np.empty((B, N, HF), dtype=np.float32)
    for core in range(NCORES):
        b = core // 2
        half = core % 2
        for q in range(IC):
            r0 = half * ROWS + q * P
            out[b, r0:r0 + P, :] = results[core][f"out{q}"].astype(np.float32)
    return out
